# revision 1
# baseline (speedup 1.0000x reference)
"""Chamfer L2 distance kernel for 8 Trainium2 NeuronCores.

Strategy (data-parallel over batch, 2 batches/core):
  For each batch and each direction (pred->target, target->pred) the device
  computes rowmin[n] = min_m H[n, m] where H = -2<x_n, y_m> + |y_m|^2 via
  K=21 bf16 matmuls (an exact hi/mid/lo bf16 decomposition of the fp32
  inputs, error ~1e-7 absolute) and a fused custom DVE min/min-reduce that
  consumes two fresh 1024-wide PSUM/SBUF tiles per pass.  The host adds the
  partition-side norms |x_n|^2 and finishes the means in fp64.

Self-contained: hardcodes B=16, N=M=4096, C=3, 8 cores.
"""

import numpy as np
import ml_dtypes

BF = ml_dtypes.bfloat16
B, N, M, C = 16, 4096, 4096, 3
NCORES = 8
BPC = B // NCORES          # batches per core
NU = BPC * 2               # (batch, orientation) units per core
K = 21                     # contraction rows (18 product terms + 3 norm rows)
NT = N // 128              # n-tiles per unit
SLOTS = NU * NT * 2        # accum slots (2 m-halves per n-tile)

_CACHE = {}


# ---------------------------------------------------------------- host prep --

def _split3(v):
    """Exact-ish 3-way bf16 decomposition: h + m + l = v + O(2^-27 |v|)."""
    h = v.astype(BF)
    r = v - h.astype(np.float64)
    m = r.astype(BF)
    r2 = r - m.astype(np.float64)
    l = r2.astype(BF)
    return h, m, l


def _build_tabs(X, Y):
    """X: (N,3) partition side, Y: (M,3) free side.
    Returns lhsT (21, N) bf16 and rhs (21, M) bf16 such that
    (lhsT.T @ rhs)[n, m] ~= -2<X_n, Y_m> + |Y_m|^2 to ~1e-7 absolute."""
    lt = np.empty((K, X.shape[0]), BF)
    rt = np.empty((K, Y.shape[0]), BF)
    Xd = X.astype(np.float64)
    Yd = -2.0 * Y.astype(np.float64)
    row = 0
    for c in range(C):
        Xh, Xm, Xl = _split3(Xd[:, c])
        Yh, Ym, Yl = _split3(Yd[:, c])
        for a, b in ((Xh, Yh), (Xh, Ym), (Xm, Yh), (Xm, Ym), (Xh, Yl), (Xl, Yh)):
            lt[row] = a
            rt[row] = b
            row += 1
    q = np.sum(Y.astype(np.float64) ** 2, axis=1)
    qh, qm, ql = _split3(q)
    ones = np.ones(X.shape[0], BF)
    for qq in (qh, qm, ql):
        lt[row] = ones
        rt[row] = qq
        row += 1
    assert row == K
    return lt, rt


# ------------------------------------------------------------- device build --

def _get_min_min_op():
    if "op" in _CACHE:
        return _CACHE["op"]
    import concourse.dve_ops as dve_ops_mod
    from concourse.dve_ops import DveOp
    from concourse.dve_spec import Spec, Src0, Src1, C0, minn, lower, _has_src1
    from concourse.dve_uop import DveOpSpec

    name = "CHAMFER_MIN_MIN_ANT"
    for op in dve_ops_mod.OPS:
        if op.name == name:
            _CACHE["op"] = op
            return op
    spec = Spec(
        body=minn(Src0, Src1),
        accum=minn,
        accum_init=C0,
        reference=lambda in0, in1, s0, s1, imm2: (
            (b := np.minimum(in0.astype(np.float32), in1.astype(np.float32))),
            np.minimum(
                b.reshape(b.shape[0], -1).min(axis=-1, keepdims=True),
                np.asarray(s0, np.float32).reshape(-1, 1),
            ),
        ),
    )
    if name not in dve_ops_mod._SUB_OPCODE_FOR_NAME:
        row = max(dve_ops_mod._SUB_OPCODE_FOR_NAME.values()) + 1
        assert row < 0x20
        dve_ops_mod._SUB_OPCODE_FOR_NAME[name] = row
    shas = {}
    for ver in ("v3", "v4"):
        try:
            s = DveOpSpec(
                name=name,
                opcode=dve_ops_mod.get_dve_sub_opcode(name),
                uops=lower(spec, ver=ver),
                rd1_en=_has_src1(spec),
            )
            shas[ver] = s.sha(ver)
        except Exception:
            pass
    op = DveOp(name, spec, False, shas)
    dve_ops_mod.OPS.append(op)
    dve_ops_mod.CUSTOM_DVE_SPECS[name] = spec
    _CACHE["op"] = op
    return op


def _build_nc():
    if "nc" in _CACHE:
        return _CACHE["nc"]
    import concourse.bacc as bacc
    import concourse.mybir as mybir
    from concourse.tile import TileContext

    MIN_MIN = _get_min_min_op()
    f32 = mybir.dt.float32
    bf16 = mybir.dt.bfloat16

    nc = bacc.Bacc(None)
    ltab = nc.dram_tensor("ltab", [NU, K, N], bf16, kind="ExternalInput")
    rtab = nc.dram_tensor("rtab", [NU, K, M], bf16, kind="ExternalInput")
    outt = nc.dram_tensor("out", [128, SLOTS], f32, kind="ExternalOutput")

    with TileContext(nc) as tc:
        with (
            tc.tile_pool(name="stage", bufs=2) as stage,
            tc.tile_pool(name="psum", bufs=2, space="PSUM") as psum,
            tc.tile_pool(name="cpp", bufs=3) as cpp,
            tc.tile_pool(name="res", bufs=1) as res,
        ):
            raw = res.tile([128, SLOTS], f32, tag="raw")
            dummy = res.tile([128, 1], f32, tag="dummy")
            for u in range(NU):
                lt = stage.tile([K, N], bf16, tag="lt")
                rt = stage.tile([K, M], bf16, tag="rt")
                nc.sync.dma_start(out=lt[:, :], in_=ltab[u])
                nc.sync.dma_start(out=rt[:, :], in_=rtab[u])
                for i in range(NT):
                    ltT = lt[:, i * 128:(i + 1) * 128]
                    for h in range(2):
                        pa = psum.tile([128, 1024], f32, tag="pa")
                        pb = psum.tile([128, 1024], f32, tag="pb")
                        base = h * 2048
                        nc.tensor.matmul(pa[:, 0:512], ltT, rt[:, base:base + 512],
                                         start=True, stop=True)
                        nc.tensor.matmul(pa[:, 512:1024], ltT, rt[:, base + 512:base + 1024],
                                         start=True, stop=True)
                        nc.tensor.matmul(pb[:, 0:512], ltT, rt[:, base + 1024:base + 1536],
                                         start=True, stop=True)
                        nc.tensor.matmul(pb[:, 512:1024], ltT, rt[:, base + 1536:base + 2048],
                                         start=True, stop=True)
                        cp = cpp.tile([128, 1024], f32, tag="cp")
                        nc.scalar.copy(out=cp[:, :], in_=pb[:, :])
                        slot = (u * NT + i) * 2 + h
                        nc.vector._custom_dve(
                            MIN_MIN,
                            out=dummy.broadcast_to(pa[:, :].shape),
                            in0=pa[:, :],
                            in1=cp[:, :],
                            s0=1.0e30,
                            accum_out=raw[:, slot:slot + 1],
                        )
            nc.sync.dma_start(out=outt[:, :], in_=raw[:, :])
    nc.compile()
    _CACHE["nc"] = nc
    return nc


# -------------------------------------------------------------------- entry --

def _prepare_inputs(pred, target):
    ltabs = np.empty((NCORES, NU, K, N), BF)
    rtabs = np.empty((NCORES, NU, K, M), BF)
    for core in range(NCORES):
        for lb in range(BPC):
            b = core * BPC + lb
            for o in range(2):
                X = pred[b] if o == 0 else target[b]
                Y = target[b] if o == 0 else pred[b]
                lt, rt = _build_tabs(X, Y)
                u = lb * 2 + o
                ltabs[core, u] = lt
                rtabs[core, u] = rt
    return ltabs, rtabs


def _postprocess(results, pred, target):
    losses = []
    for core in range(NCORES):
        out = np.asarray(results[core]["out"])  # (128, SLOTS)
        for lb in range(BPC):
            b = core * BPC + lb
            total = 0.0
            for o in range(2):
                u = lb * 2 + o
                sl = out[:, u * (NT * 2):(u + 1) * (NT * 2)]
                rowmin = sl.reshape(128, NT, 2).min(axis=2)      # (p, i)
                rowmin = rowmin.T.reshape(-1)                     # n = i*128 + p
                X = pred[b] if o == 0 else target[b]
                s2 = np.sum(X.astype(np.float64) ** 2, axis=1)
                total += (s2 + rowmin).mean()
            losses.append(total)
    return np.float32(np.mean(losses))


def _run(pred, target, trace=False):
    from concourse.bass_utils import run_bass_kernel_spmd

    pred = np.asarray(pred, dtype=np.float32)
    target = np.asarray(target, dtype=np.float32)
    assert pred.shape == (B, N, C) and target.shape == (B, M, C)
    ltabs, rtabs = _prepare_inputs(pred, target)
    nc = _build_nc()
    in_maps = [{"ltab": ltabs[c], "rtab": rtabs[c]} for c in range(NCORES)]
    res = run_bass_kernel_spmd(nc, in_maps, core_ids=list(range(NCORES)), trace=trace)
    return _postprocess(res.results, pred, target), res


def kernel(pred, target):
    loss, _ = _run(pred, target, trace=False)
    return loss
